# revision 84
# baseline (speedup 1.0000x reference)
"""Trainium2 Bass kernel for NonlocalSingleBlock (B=8, C=256, N=2048).

Sharding: data-parallel over batch B across the 8 NeuronCores (one batch
element per core). Per core:
  Q = wq@x+bq, K = wk@x+bk (natural [C,N] layout)
  VT = (wv@x+bv)^T computed directly as x^T @ wv^T (no on-chip transpose)
  S^T tiles [m,n] = K^T Q; scaled by host-pretransposed beta^T; exp on ACT
  (softmax without max-subtraction -- |S*beta| is bounded well under exp
  overflow)
  Softmax row-sums: split between the PE (ones-matmul over a subset of the
  exp'd score tiles) and the otherwise-idle GPSIMD/Pool engine (elementwise
  accumulation chain over the rest, finished by one f32r ones-matmul).
  message = VT-stationary matmuls; MLP with BatchNorm folded into the conv
  weights host-side; residual add.
Attention/MLP matmuls run in bf16; QKV projections stay f32r for accuracy.
beta^T streams as fp16 (half the DMA of f32 at ~1e-4 relative error -- the
fp32 max-|S*beta| exponent error this introduces is ~0.03).
All weights are packed into one DRAM image (single DMA). The softmax
pipeline is 4-deep double-buffered (PSUM: st x4, msg x2 banks, sums, proj)
so PE overlaps the DVE multiply and ACT exp stages.
"""

import numpy as np
import ml_dtypes

import concourse.bass as bass
import concourse.bacc as bacc
import concourse.tile as tile
import concourse.mybir as mybir
import concourse.bass_utils as bass_utils

B, C, N = 8, 256, 2048
EPS = 1e-5
F32 = mybir.dt.float32
F32R = mybir.dt.float32r
BF16 = mybir.dt.bfloat16
F16 = mybir.dt.float16
NB = 4          # n-blocks per core
BLK = N // NB   # 512 query columns per block
MCH = N // 128  # 16 key chunks of 128

DT_S = BF16     # Q/K/S^T matmul dtype
DT_V = BF16     # es/VT path
DT_M = BF16     # MLP path
DT_X = F32R     # x for QKV projections

_CACHE = {}


def _np_of(dt):
    return ml_dtypes.bfloat16 if dt == BF16 else np.float32


def _pack_layout():
    """Column layout of the packed weight images. Returns (lay4, lay16, n4,
    n16) where lay*[name] = (start, ncols). wqT/wkT lead the bf16 image so
    the startup head-DMA covers exactly what the first matmuls need."""
    entries = [
        ("wqT", 512, BF16), ("wkT", 512, BF16), ("wvT", 512, BF16),
        ("w1T", 256, DT_M), ("w2T", 128, DT_M), ("w3T", 256, DT_M),
        ("ones", 128, DT_V),
        ("bias", 8, F32R), ("bv", 256, F32R), ("ones32", 128, F32R),
        ("ident", 128, F32R),
    ]
    lay4, lay16 = {}, {}
    n4 = n16 = 0
    for name, ncols, dt in entries:
        if dt == BF16:
            lay16[name] = (n16, ncols)
            n16 += ncols
        else:
            lay4[name] = (n4, ncols)
            n4 += ncols
    return lay4, lay16, n4, max(n16, 1)


def build_nc(loop_iters=None, beta_dt=F16, pool_sums=12, qk_bias="mixed",
             bt_bufs=8, outp_bufs=4, sums_skew=2):
    nc = bacc.Bacc("TRN2", target_bir_lowering=False, debug=False)

    d = {}
    # "x" carries the host-prefolded residual x + b3 (f32 bits read as f32r)
    d["x"] = nc.dram_tensor("x", [C, N], F32R, kind="ExternalInput")
    d["x16"] = nc.dram_tensor("x16", [C, N], BF16, kind="ExternalInput")
    d["betaT"] = nc.dram_tensor("betaT", [N, N], beta_dt, kind="ExternalInput")
    lay4, lay16, n4, n16 = _pack_layout()
    d["wpack"] = nc.dram_tensor("wpack", [128, n4], F32R, kind="ExternalInput")
    if lay16:
        d["wpack16"] = nc.dram_tensor("wpack16", [128, n16], BF16,
                                      kind="ExternalInput")
    d["out"] = nc.dram_tensor("out", [C, N], F32, kind="ExternalOutput")
    if loop_iters == "rt":
        # runtime trip count: one executable serves every K, so the
        # per-executable dispatch floor cancels exactly in differentials
        d["niter"] = nc.dram_tensor("niter", [1, 1], mybir.dt.int32,
                                    kind="ExternalInput")

    from contextlib import ExitStack, nullcontext
    with tile.TileContext(nc) as tc, ExitStack() as ctx:
        P = {}
        P["consts"] = ctx.enter_context(tc.tile_pool(name="consts", bufs=1))
        P["big"] = ctx.enter_context(tc.tile_pool(name="big", bufs=1))
        P["bt"] = ctx.enter_context(tc.tile_pool(name="bt", bufs=bt_bufs))
        P["es"] = ctx.enter_context(tc.tile_pool(name="es", bufs=10))
        P["sbm"] = ctx.enter_context(tc.tile_pool(name="sbm", bufs=4))
        P["acc"] = ctx.enter_context(tc.tile_pool(name="acc", bufs=2))
        P["msgsb"] = ctx.enter_context(tc.tile_pool(name="msgsb", bufs=2))
        P["recip"] = ctx.enter_context(tc.tile_pool(name="recip", bufs=2))
        P["h"] = ctx.enter_context(tc.tile_pool(name="h", bufs=2))
        P["outp"] = ctx.enter_context(tc.tile_pool(name="outp", bufs=outp_bufs))
        # PSUM banks: st 3x1 + msg 2x2 + proj 1 = 8 (softmax sums share the
        # proj bank -- they complete right before the MLP needs it)
        P["st"] = ctx.enter_context(tc.tile_pool(name="st", bufs=3, space="PSUM"))
        P["msg"] = ctx.enter_context(tc.tile_pool(name="msg", bufs=2, space="PSUM"))
        P["proj"] = ctx.enter_context(tc.tile_pool(name="proj", bufs=1, space="PSUM"))

        cst = _load_consts(nc, P, d)
        if loop_iters == "rt":
            nit_sb = P["consts"].tile([1, 1], mybir.dt.int32, name="nit_sb")
            nc.sync.dma_start(out=nit_sb, in_=d["niter"].ap())
            regs = []
            for eng in nc.engines.values():
                r = eng.alloc_register(f"niter_{eng.engine.name}")
                eng.reg_load(r, nit_sb)
                regs.append(r)
            bound = nc.snap(bass.RegisterHandles(regs), min_val=1,
                            max_val=1 << 20)
            loop_cm = tc.For_i(0, bound, 1)
        else:
            loop_cm = tc.For_i(0, loop_iters, 1) if loop_iters \
                else nullcontext()
        with loop_cm:
            _emit_body(nc, tc, P, d, cst, beta_dt, pool_sums, qk_bias,
                       sums_skew)

    nc.compile()
    return nc


def _load_consts(nc, P, d):
    consts = P["consts"]
    lay4, lay16, n4, n16 = _pack_layout()
    cst = {}
    # Startup DMAs fan out across queues so the serial per-DMA overhead
    # (~1.4us each) isn't stacked on one queue: scalar gets the wqT/wkT head
    # (first matmuls) then the f32 x; sync gets the small f32 pack (biases)
    # and the rest of the bf16 pack (wvT for the VT phase) before the beta
    # stream; the x16 chunks ride the vector queue.
    wp16 = consts.tile([128, n16], BF16, name="wp16_sb")
    head = min(1024, n16)
    nc.scalar.dma_start(out=wp16[:, 0:head], in_=d["wpack16"].ap()[:, 0:head])
    wp4 = consts.tile([128, n4], F32R, name="wp4_sb")
    cst["_deferred"] = lambda: (
        nc.scalar.dma_start(out=wp4, in_=d["wpack"].ap()),
        nc.scalar.dma_start(out=wp16[:, head:n16],
                            in_=d["wpack16"].ap()[:, head:n16])
        if n16 > head else None,
    )

    def sl(name):
        lay, t = (lay4, wp4) if name in lay4 else (lay16, wp16)
        off, ncols = lay[name]
        return t[:, off:off + ncols]

    for nm in ("wqT", "wkT", "wvT"):
        cst[nm] = sl(nm).rearrange("p (t o) -> p t o", t=2)
    cst["w1T"] = sl("w1T").rearrange("p (t o) -> p t o", t=2)
    cst["w2T"] = sl("w2T")
    cst["w3T"] = sl("w3T")
    b = sl("bias").bitcast(F32)
    cst["bq"] = b[:, 0:2]
    cst["bk"] = b[:, 2:4]
    cst["b3"] = b[:, 4:6]
    cst["b1"] = b[:, 6:7]
    cst["b2"] = b[:, 7:8]
    cst["bv"] = sl("bv").bitcast(F32)
    cst["ones"] = sl("ones")
    cst["ones32"] = sl("ones32")
    cst["ident"] = sl("ident")
    return cst


def _emit_body(nc, tc, P, d, cst, beta_dt, pool_sums, qk_bias, sums_skew=2):
    AF = mybir.ActivationFunctionType
    OP = mybir.AluOpType
    x_d, betaT_d, out_d = d["x"], d["betaT"], d["out"]

    # First pool_sums mi chunks' es tiles are accumulated on the Pool engine
    # (prefix, so the chain and its closing f32r ones-matmul finish mid-block
    # instead of serializing the block tail); the rest go through the PE
    # ones-matmul as before.
    pool_set = set(range(pool_sums))
    pe_set = [mi for mi in range(MCH) if mi not in pool_set]

    # ---- x16 (4 DMAs on the scalar queue, parallel to the wpack DMAs on
    # the sync queue, so the first QK matmuls start early). The f32 x for
    # the residual follows on the sync queue -- it is not needed until the
    # first MLP, ~20us in. ----
    # x16 chunk 0 rides the sync queue in parallel with the wqT/wkT head on
    # the scalar queue, so the first matmul can start after ~one DMA latency;
    # the remaining chunks stream on scalar behind the head.
    x16_sb = P["big"].tile([128, 2, N], BF16, tag="x16", name="x16_sb")
    x16_re = d["x16"].ap().rearrange("(t p) n -> p t n", p=128)
    qs = slice(0, N // 4)
    nc.scalar.dma_start(out=x16_sb[:, :, qs], in_=x16_re[:, :, qs])
    cst["_deferred"]()  # small f32 pack + rest of bf16 pack, needed ~5us in
    for q in range(1, 4):
        qs = slice(q * (N // 4), (q + 1) * (N // 4))
        nc.scalar.dma_start(out=x16_sb[:, :, qs], in_=x16_re[:, :, qs])
    x_sb = P["big"].tile([128, 2, N], F32R, tag="x", name="x_sb")
    nc.scalar.dma_start(out=x_sb,
                        in_=x_d.ap().rearrange("(t p) n -> p t n", p=128))

    # ---- Q, K: [co, n] = sum_ci wT[ci, co] x[ci, n] + b[co] ----
    q_sb = P["big"].tile([128, 2, N], DT_S, tag="q", name="q_sb")
    k_sb = P["big"].tile([128, 2, N], DT_S, tag="k", name="k_sb")
    qk_rot = ["st", "st", "st", "msg", "msg", "proj"]
    qk_i = 0
    for w_sb, b_sb, dst in ((cst["wqT"], cst["bq"], q_sb), (cst["wkT"], cst["bk"], k_sb)):
        for co in range(2):
            for nb in range(NB):
                rtag = qk_rot[qk_i % len(qk_rot)]
                qk_i += 1
                ps = P[rtag].tile([128, BLK], F32, tag=rtag, name="qk_ps")
                nsl = slice(nb * BLK, (nb + 1) * BLK)
                for ci in range(2):
                    nc.tensor.matmul(
                        ps,
                        w_sb[:, ci, co * 128:(co + 1) * 128],
                        x16_sb[:, ci, nsl],
                        start=(ci == 0), stop=(ci == 1),
                    )
                dst_ap = dst[:, co, nsl]
                if nb % 2 == 0:
                    nc.scalar.add(dst_ap, ps, b_sb[:, co:co + 1])
                else:
                    nc.vector.tensor_scalar_add(dst_ap, ps, b_sb[:, co:co + 1])

    # ---- VT[m, c] = sum_ci x[ci, m]^T wvT[ci, c] + bv ----
    vt_sb = P["big"].tile([128, MCH, C], DT_V, tag="vt", name="vt_sb")
    bvap = cst["bv"]
    bv_b = bass.AP(tensor=bvap.tensor, offset=bvap.offset,
                   ap=[bvap.ap[0], [0, 2], bvap.ap[1]])
    vt_rot = ["st", "st", "msg", "proj"]
    for mp in range(MCH // 2):
        rtag = vt_rot[mp % 4]
        ps = P[rtag].tile([128, 2, C], F32, tag=rtag, name="vt_ps")
        for j in range(2):
            mi = 2 * mp + j
            for ci in range(2):
                nc.tensor.matmul(
                    ps[:, j, :],
                    x16_sb[:, ci, mi * 128:(mi + 1) * 128],
                    cst["wvT"][:, ci, :],
                    start=(ci == 0), stop=(ci == 1),
                )
        nc.vector.tensor_add(vt_sb[:, 2 * mp:2 * mp + 2, :], ps, bv_b)

    # ---- attention + MLP per n-block, software-pipelined: block nb's MLP
    # (recip/norm on DVE, h-chain on PE) is emitted a few mi-iterations into
    # block nb+1's attention so the in-order PE is never parked on the DVE
    # tail at a block boundary. ----
    # The previous block's MLP is emitted whole at this block's mi=3
    # (spreading the stages across more iterations measured WORSE on HW).
    MLP_DELAY_MI = 3
    pending_mlp = [None]

    def emit_mlp_stage(mi=None):
        stages = pending_mlp[0]
        if not stages:
            return
        if mi is None or mi == MLP_DELAY_MI:
            while stages:
                stages.pop(0)()
            pending_mlp[0] = None

    for nb in range(NB):
        nsl = slice(nb * BLK, (nb + 1) * BLK)
        msg_ps = P["msg"].tile([128, 2 * BLK], F32, tag="msg", name="msg_ps")
        sums_ps = None
        bts = {}
        for mp in range(MCH // 2):
            bt = P["bt"].tile([128, 2, BLK], beta_dt, tag="bt", name="bt_sb")
            nc.sync.dma_start(
                out=bt,
                in_=betaT_d.ap()[2 * mp * 128:(2 * mp + 2) * 128, nsl]
                    .rearrange("(a p) n -> p a n", p=128))
            bts[mp] = bt
        acc = P["acc"].tile([128, BLK], F32R, tag="acc", name="acc_sb") \
            if pool_set else None
        pool_pending = []
        acc_started = False

        def drain_pool(force=False):
            nonlocal acc_started, pool_pending
            while pool_pending:
                if not acc_started:
                    if len(pool_pending) >= 2:
                        a, b2 = pool_pending[0], pool_pending[1]
                        pool_pending = pool_pending[2:]
                        nc.gpsimd.tensor_tensor(out=acc, in0=a, in1=b2, op=OP.add)
                        acc_started = True
                    elif force:
                        a = pool_pending.pop(0)
                        nc.gpsimd.tensor_copy(out=acc, in_=a)
                        acc_started = True
                    else:
                        return
                else:
                    a = pool_pending.pop(0)
                    nc.gpsimd.tensor_tensor(out=acc, in0=acc, in1=a, op=OP.add)

        # The es-consuming PE matmuls (msg, pe-sums) are emitted one mi later
        # than their producer: the engine wait-queue is only 4 deep, and
        # stacking msg x2 + sums behind a just-issued es stalls the PE
        # sequencer; with a 1-iteration skew the es has ~1us of slack.
        es_pending = []
        sums_pending = []

        def emit_sums(es, mi):
            nonlocal sums_ps
            if sums_ps is None:
                sums_ps = P["proj"].tile([128, BLK], F32, tag="proj",
                                         name="sums_ps")
            nc.tensor.matmul(sums_ps, cst["ones"], es,
                             start=(mi == pe_set[0]),
                             stop=(not pool_set and mi == pe_set[-1]))

        def emit_consumers(es, mi):
            nc.tensor.matmul(msg_ps[:, 0:BLK], vt_sb[:, mi, 0:128], es,
                             start=(mi == 0), stop=(mi == MCH - 1))
            nc.tensor.matmul(msg_ps[:, BLK:2 * BLK], vt_sb[:, mi, 128:256], es,
                             start=(mi == 0), stop=(mi == MCH - 1))
            if mi not in pool_set:
                sums_pending.append((es, mi))
                if len(sums_pending) > sums_skew - 2:
                    emit_sums(*sums_pending.pop(0))

        for mi in range(MCH):
            emit_mlp_stage(mi)
            msl = slice(mi * 128, (mi + 1) * 128)
            st = P["st"].tile([128, BLK], F32, tag="st", name="st_ps")
            for ci in range(2):
                nc.tensor.matmul(
                    st,
                    k_sb[:, ci, msl],
                    q_sb[:, ci, nsl],
                    start=(ci == 0), stop=(ci == 1),
                )
            sbm = P["sbm"].tile([128, BLK], F32, tag="sbm", name="sbm_sb")
            nc.vector.tensor_mul(sbm, st, bts[mi // 2][:, mi % 2, :])
            es = P["es"].tile([128, BLK], DT_V, tag="es", name="es_sb")
            nc.scalar.activation(es, sbm, AF.Exp)
            es_pending.append((es, mi))
            if len(es_pending) > 2:
                emit_consumers(*es_pending.pop(0))
            if mi in pool_set:
                pool_pending.append(es)
                drain_pool()
                if mi == pool_sums - 1:
                    drain_pool(force=True)
        while es_pending:
            emit_consumers(*es_pending.pop(0))
        while sums_pending:
            emit_sums(*sums_pending.pop(0))
        # fold the Pool accumulator into the sums bank last: by the time the
        # in-order PE reaches this, the Pool chain has long finished
        if pool_set:
            if sums_ps is None:
                sums_ps = P["proj"].tile([128, BLK], F32, tag="proj",
                                         name="sums_ps")
            nc.tensor.matmul(sums_ps, cst["ones32"], acc,
                             start=(not pe_set), stop=True)
        def make_mlp(nsl, msg_ps, sums_ps, splits=1):
            # MLP: h1 = relu(w1f@msg+b1f); h2 = relu(w2f@h1+b2f);
            # out = (x+b3) + w3@h2. For the final block (nothing left to
            # overlap with), run in column halves pipelined across the
            # proj and msg PSUM banks to hide the relu handoffs.
            W = BLK // splits
            hs = [(s, slice(s * W, (s + 1) * W),
                   "proj" if s % 2 == 0 else "msg") for s in range(splits)]
            st8 = {}

            def stage_norm():
                recip = P["recip"].tile([128, BLK], F32, tag="recip",
                                        name="recip_sb")
                nc.vector.reciprocal(recip, sums_ps)
                msg_sb = P["msgsb"].tile([128, 2, BLK], DT_M, tag="msgsb",
                                         name="msg_sb")
                nc.vector.tensor_mul(msg_sb[:, 0, :], msg_ps[:, 0:BLK], recip)
                nc.vector.tensor_mul(msg_sb[:, 1, :], msg_ps[:, BLK:2 * BLK],
                                     recip)
                st8["msg_sb"] = msg_sb

            def stage_h1():
                for s, csl, pool in hs:
                    h1p = P[pool].tile([128, W], F32, tag=pool, name="h1_ps")
                    for ci in range(2):
                        nc.tensor.matmul(h1p, cst["w1T"][:, ci, :],
                                         st8["msg_sb"][:, ci, csl],
                                         start=(ci == 0), stop=(ci == 1))
                    h1 = P["h"].tile([128, W], DT_M, tag=f"h1{s}",
                                     name="h1_sb")
                    nc.scalar.activation(h1, h1p, AF.Relu,
                                         bias=cst["b1"][:, 0:1])
                    st8[f"h1{s}"] = h1

            def stage_h2():
                for s, csl, pool in hs:
                    h2p = P[pool].tile([128, W], F32, tag=pool, name="h2_ps")
                    nc.tensor.matmul(h2p, cst["w2T"], st8[f"h1{s}"],
                                     start=True, stop=True)
                    h2 = P["h"].tile([128, W], DT_M, tag=f"h2{s}",
                                     name="h2_sb")
                    nc.scalar.activation(h2, h2p, AF.Relu,
                                         bias=cst["b2"][:, 0:1])
                    st8[f"h2{s}"] = h2

            def stage_h3():
                base = nsl.start
                for s, csl, pool in hs:
                    for co in range(2):
                        h3p = P[pool].tile([128, W], F32, tag=pool,
                                           name="h3_ps")
                        xsl = slice(base + csl.start, base + csl.stop)
                        # seed the accumulator with the prefolded residual
                        # (x + b3) via an identity matmul, then accumulate
                        # w3@h2 on top
                        nc.tensor.matmul(h3p, cst["ident"],
                                         x_sb[:, co, xsl],
                                         start=True, stop=False)
                        nc.tensor.matmul(h3p,
                                         cst["w3T"][:, co * 128:(co + 1) * 128],
                                         st8[f"h2{s}"], start=False, stop=True,
                                         skip_group_check=True)
                        ob = P["outp"].tile([128, W], F32, tag="ob",
                                            name="ob_sb")
                        nc.scalar.activation(ob, h3p, AF.Copy)
                        nc.sync.dma_start(
                            out=out_d.ap()[co * 128:(co + 1) * 128, xsl],
                            in_=ob)

            return [stage_norm, stage_h1, stage_h2, stage_h3]

        pending_mlp[0] = make_mlp(nsl, msg_ps, sums_ps,
                                  splits=(2 if nb == NB - 1 else 1))
    emit_mlp_stage()


def _prep_host(inputs, beta_dt=F16, **_kw):
    """Fold BN into conv weights, pre-transpose weights, build per-core maps."""
    f = np.float32
    wq, bq = inputs["wq"].astype(f), inputs["bq"].astype(f)
    wk, bk = inputs["wk"].astype(f), inputs["bk"].astype(f)
    wv, bv = inputs["wv"].astype(f), inputs["bv"].astype(f)
    inv1 = inputs["g1"] / np.sqrt(inputs["v1"] + EPS)
    w1f = (inputs["w1"] * inv1[:, None]).astype(f)
    b1f = (inputs["b1"] * inv1 + inputs["be1"] - inputs["m1"] * inv1).astype(f)
    inv2 = inputs["g2"] / np.sqrt(inputs["v2"] + EPS)
    w2f = (inputs["w2"] * inv2[:, None]).astype(f)
    b2f = (inputs["b2"] * inv2 + inputs["be2"] - inputs["m2"] * inv2).astype(f)
    w3, b3 = inputs["w3"].astype(f), inputs["b3"].astype(f)

    def fold2(wT):  # [256, X] -> [128, 2*X] with t-major columns
        X = wT.shape[1]
        return wT.reshape(2, 128, X).transpose(1, 0, 2).reshape(128, 2 * X)

    lay4, lay16, n4, n16 = _pack_layout()
    pack4 = np.zeros((128, n4), dtype=f)
    pack16 = np.zeros((128, n16), dtype=ml_dtypes.bfloat16)

    def put(name, arr):
        if name in lay4:
            off, ncols = lay4[name]
            pack4[:, off:off + ncols] = arr
        else:
            off, ncols = lay16[name]
            pack16[:, off:off + ncols] = arr.astype(ml_dtypes.bfloat16)

    put("wqT", fold2(wq.T))
    put("wkT", fold2(wk.T))
    put("wvT", fold2(wv.T))
    put("w1T", fold2(w1f.T))
    put("w2T", w2f.T)
    put("w3T", w3.T)
    bias_cols = np.zeros((128, 8), dtype=f)
    bias_cols[:, 0:2] = bq.reshape(2, 128).T
    bias_cols[:, 2:4] = bk.reshape(2, 128).T
    bias_cols[:, 6] = b1f
    bias_cols[:, 7] = b2f
    put("bias", bias_cols)
    put("bv", np.tile(bv, (128, 1)))
    put("ones", np.ones((128, 128), dtype=f))
    put("ones32", np.ones((128, 128), dtype=f))
    put("ident", np.eye(128, dtype=f))
    shared = {"wpack": pack4}
    if lay16:
        shared["wpack16"] = pack16
    x = np.asarray(inputs["cors_feature"], dtype=f)
    beta = np.asarray(inputs["beta_attention"], dtype=f)
    np_beta = {F16: np.float16, BF16: ml_dtypes.bfloat16, F32: f}[beta_dt]
    in_maps = []
    for b in range(B):
        m = dict(shared)
        m["x"] = np.ascontiguousarray(x[b] + b3[:, None])
        m["x16"] = np.ascontiguousarray(x[b]).astype(ml_dtypes.bfloat16)
        m["betaT"] = np.ascontiguousarray(beta[b].T).astype(np_beta)
        in_maps.append(m)
    return in_maps


def kernel(**inputs) -> np.ndarray:
    if "nc" not in _CACHE:
        _CACHE["nc"] = build_nc()
    nc = _CACHE["nc"]
    in_maps = _prep_host(inputs)
    res = bass_utils.run_bass_kernel_spmd(
        nc, in_maps, core_ids=list(range(B)), trace=False)
    out = np.stack([res.results[b]["out"] for b in range(B)], axis=0)
    return out.astype(np.float32)


# revision 88
# speedup vs baseline: 1.0405x; 1.0405x over previous
"""Trainium2 Bass kernel for NonlocalSingleBlock (B=8, C=256, N=2048).

Sharding: data-parallel over batch B across the 8 NeuronCores (one batch
element per core). Per core:
  Q = wq@x+bq, K = wk@x+bk (natural [C,N] layout)
  VT = (wv@x+bv)^T computed directly as x^T @ wv^T (no on-chip transpose)
  S^T tiles [m,n] = K^T Q; scaled by host-pretransposed beta^T; exp on ACT
  (softmax without max-subtraction -- |S*beta| is bounded well under exp
  overflow)
  Softmax row-sums: split between the PE (ones-matmul over a subset of the
  exp'd score tiles) and the otherwise-idle GPSIMD/Pool engine (elementwise
  accumulation chain over the rest, finished by one f32r ones-matmul).
  message = VT-stationary matmuls; MLP with BatchNorm folded into the conv
  weights host-side; residual add.
Attention/MLP matmuls run in bf16; QKV projections stay f32r for accuracy.
beta^T streams as fp16 (half the DMA of f32 at ~1e-4 relative error -- the
fp32 max-|S*beta| exponent error this introduces is ~0.03).
All weights are packed into one DRAM image (single DMA). The softmax
pipeline is 4-deep double-buffered (PSUM: st x4, msg x2 banks, sums, proj)
so PE overlaps the DVE multiply and ACT exp stages.
"""

import numpy as np
import ml_dtypes

import concourse.bass as bass
import concourse.bacc as bacc
import concourse.tile as tile
import concourse.mybir as mybir
import concourse.bass_utils as bass_utils

B, C, N = 8, 256, 2048
EPS = 1e-5
F32 = mybir.dt.float32
F32R = mybir.dt.float32r
BF16 = mybir.dt.bfloat16
F16 = mybir.dt.float16
NB = 4          # n-blocks per core
BLK = N // NB   # 512 query columns per block
MCH = N // 128  # 16 key chunks of 128

DT_S = BF16     # Q/K/S^T matmul dtype
DT_V = BF16     # es/VT path
DT_M = BF16     # MLP path
DT_X = F32R     # x for QKV projections

_CACHE = {}


def _np_of(dt):
    return ml_dtypes.bfloat16 if dt == BF16 else np.float32


def _pack_layout():
    """Column layout of the packed weight images. Returns (lay4, lay16, n4,
    n16) where lay*[name] = (start, ncols). wqT/wkT lead the bf16 image so
    the startup head-DMA covers exactly what the first matmuls need."""
    entries = [
        ("wqT", 512, BF16), ("wkT", 512, BF16), ("wvT", 512, BF16),
        ("w1T", 256, DT_M), ("w2T", 128, DT_M), ("w3T", 256, DT_M),
        ("ones", 128, DT_V),
        ("bias", 8, F32R), ("bv", 256, F32R), ("ones32", 128, F32R),
        ("ident", 128, F32R),
    ]
    lay4, lay16 = {}, {}
    n4 = n16 = 0
    for name, ncols, dt in entries:
        if dt == BF16:
            lay16[name] = (n16, ncols)
            n16 += ncols
        else:
            lay4[name] = (n4, ncols)
            n4 += ncols
    return lay4, lay16, n4, max(n16, 1)


def build_nc(loop_iters=None, beta_dt=F16, pool_sums=12, qk_bias="mixed",
             bt_bufs=8, outp_bufs=4, sums_skew=2, es_skew=2):
    nc = bacc.Bacc("TRN2", target_bir_lowering=False, debug=False)

    d = {}
    # "x" carries the host-prefolded residual x + b3 (f32 bits read as f32r)
    d["x"] = nc.dram_tensor("x", [C, N], F32R, kind="ExternalInput")
    d["x16"] = nc.dram_tensor("x16", [C, N], BF16, kind="ExternalInput")
    d["betaT"] = nc.dram_tensor("betaT", [N, N], beta_dt, kind="ExternalInput")
    lay4, lay16, n4, n16 = _pack_layout()
    d["wpack"] = nc.dram_tensor("wpack", [128, n4], F32R, kind="ExternalInput")
    if lay16:
        d["wpack16"] = nc.dram_tensor("wpack16", [128, n16], BF16,
                                      kind="ExternalInput")
    d["out"] = nc.dram_tensor("out", [C, N], F32, kind="ExternalOutput")
    if loop_iters == "rt":
        # runtime trip count: one executable serves every K, so the
        # per-executable dispatch floor cancels exactly in differentials
        d["niter"] = nc.dram_tensor("niter", [1, 1], mybir.dt.int32,
                                    kind="ExternalInput")

    from contextlib import ExitStack, nullcontext
    with tile.TileContext(nc) as tc, ExitStack() as ctx:
        P = {}
        P["consts"] = ctx.enter_context(tc.tile_pool(name="consts", bufs=1))
        P["big"] = ctx.enter_context(tc.tile_pool(name="big", bufs=1))
        P["bt"] = ctx.enter_context(tc.tile_pool(name="bt", bufs=bt_bufs))
        P["es"] = ctx.enter_context(tc.tile_pool(name="es", bufs=10))
        P["sbm"] = ctx.enter_context(tc.tile_pool(name="sbm", bufs=4))
        P["acc"] = ctx.enter_context(tc.tile_pool(name="acc", bufs=2))
        P["msgsb"] = ctx.enter_context(tc.tile_pool(name="msgsb", bufs=2))
        P["recip"] = ctx.enter_context(tc.tile_pool(name="recip", bufs=2))
        P["h"] = ctx.enter_context(tc.tile_pool(name="h", bufs=2))
        P["outp"] = ctx.enter_context(tc.tile_pool(name="outp", bufs=outp_bufs))
        # PSUM banks: st 3x1 + msg 2x2 + proj 1 = 8 (softmax sums share the
        # proj bank -- they complete right before the MLP needs it)
        P["st"] = ctx.enter_context(tc.tile_pool(name="st", bufs=3, space="PSUM"))
        P["msg"] = ctx.enter_context(tc.tile_pool(name="msg", bufs=2, space="PSUM"))
        P["proj"] = ctx.enter_context(tc.tile_pool(name="proj", bufs=1, space="PSUM"))

        cst = _load_consts(nc, P, d)
        if loop_iters == "rt":
            nit_sb = P["consts"].tile([1, 1], mybir.dt.int32, name="nit_sb")
            nc.sync.dma_start(out=nit_sb, in_=d["niter"].ap())
            regs = []
            for eng in nc.engines.values():
                r = eng.alloc_register(f"niter_{eng.engine.name}")
                eng.reg_load(r, nit_sb)
                regs.append(r)
            bound = nc.snap(bass.RegisterHandles(regs), min_val=1,
                            max_val=1 << 20)
            loop_cm = tc.For_i(0, bound, 1)
        else:
            loop_cm = tc.For_i(0, loop_iters, 1) if loop_iters \
                else nullcontext()
        with loop_cm:
            _emit_body(nc, tc, P, d, cst, beta_dt, pool_sums, qk_bias,
                       sums_skew, es_skew)

    nc.compile()
    return nc


def _load_consts(nc, P, d):
    consts = P["consts"]
    lay4, lay16, n4, n16 = _pack_layout()
    cst = {}
    # Startup DMAs fan out across queues so the serial per-DMA overhead
    # (~1.4us each) isn't stacked on one queue: scalar gets the wqT/wkT head
    # (first matmuls) then the f32 x; sync gets the small f32 pack (biases)
    # and the rest of the bf16 pack (wvT for the VT phase) before the beta
    # stream; the x16 chunks ride the vector queue.
    wp16 = consts.tile([128, n16], BF16, name="wp16_sb")
    head = min(1024, n16)
    nc.scalar.dma_start(out=wp16[:, 0:head], in_=d["wpack16"].ap()[:, 0:head])
    wp4 = consts.tile([128, n4], F32R, name="wp4_sb")
    cst["_deferred"] = lambda: (
        nc.scalar.dma_start(out=wp4, in_=d["wpack"].ap()),
        nc.scalar.dma_start(out=wp16[:, head:n16],
                            in_=d["wpack16"].ap()[:, head:n16])
        if n16 > head else None,
    )

    def sl(name):
        lay, t = (lay4, wp4) if name in lay4 else (lay16, wp16)
        off, ncols = lay[name]
        return t[:, off:off + ncols]

    for nm in ("wqT", "wkT", "wvT"):
        cst[nm] = sl(nm).rearrange("p (t o) -> p t o", t=2)
    cst["w1T"] = sl("w1T").rearrange("p (t o) -> p t o", t=2)
    cst["w2T"] = sl("w2T")
    cst["w3T"] = sl("w3T")
    b = sl("bias").bitcast(F32)
    cst["bq"] = b[:, 0:2]
    cst["bk"] = b[:, 2:4]
    cst["b3"] = b[:, 4:6]
    cst["b1"] = b[:, 6:7]
    cst["b2"] = b[:, 7:8]
    cst["bv"] = sl("bv").bitcast(F32)
    cst["ones"] = sl("ones")
    cst["ones32"] = sl("ones32")
    cst["ident"] = sl("ident")
    return cst


def _emit_body(nc, tc, P, d, cst, beta_dt, pool_sums, qk_bias, sums_skew=2,
               es_skew=2):
    AF = mybir.ActivationFunctionType
    OP = mybir.AluOpType
    x_d, betaT_d, out_d = d["x"], d["betaT"], d["out"]

    # First pool_sums mi chunks' es tiles are accumulated on the Pool engine
    # (prefix, so the chain and its closing f32r ones-matmul finish mid-block
    # instead of serializing the block tail); the rest go through the PE
    # ones-matmul as before.
    pool_set = set(range(pool_sums))
    pe_set = [mi for mi in range(MCH) if mi not in pool_set]

    # ---- x16 (4 DMAs on the scalar queue, parallel to the wpack DMAs on
    # the sync queue, so the first QK matmuls start early). The f32 x for
    # the residual follows on the sync queue -- it is not needed until the
    # first MLP, ~20us in. ----
    # x16 chunk 0 rides the sync queue in parallel with the wqT/wkT head on
    # the scalar queue, so the first matmul can start after ~one DMA latency;
    # the remaining chunks stream on scalar behind the head.
    x16_sb = P["big"].tile([128, 2, N], BF16, tag="x16", name="x16_sb")
    x16_re = d["x16"].ap().rearrange("(t p) n -> p t n", p=128)
    qs = slice(0, N // 4)
    nc.scalar.dma_start(out=x16_sb[:, :, qs], in_=x16_re[:, :, qs])
    cst["_deferred"]()  # small f32 pack + rest of bf16 pack, needed ~5us in
    for q in range(1, 4):
        qs = slice(q * (N // 4), (q + 1) * (N // 4))
        nc.scalar.dma_start(out=x16_sb[:, :, qs], in_=x16_re[:, :, qs])
    x_sb = P["big"].tile([128, 2, N], F32R, tag="x", name="x_sb")
    nc.scalar.dma_start(out=x_sb,
                        in_=x_d.ap().rearrange("(t p) n -> p t n", p=128))

    # ---- Q, K: [co, n] = sum_ci wT[ci, co] x[ci, n] + b[co] ----
    q_sb = P["big"].tile([128, 2, N], DT_S, tag="q", name="q_sb")
    k_sb = P["big"].tile([128, 2, N], DT_S, tag="k", name="k_sb")
    qk_rot = ["st", "st", "st", "msg", "msg", "proj"]
    qk_i = 0
    for w_sb, b_sb, dst in ((cst["wqT"], cst["bq"], q_sb), (cst["wkT"], cst["bk"], k_sb)):
        for co in range(2):
            for nb in range(NB):
                rtag = qk_rot[qk_i % len(qk_rot)]
                qk_i += 1
                ps = P[rtag].tile([128, BLK], F32, tag=rtag, name="qk_ps")
                nsl = slice(nb * BLK, (nb + 1) * BLK)
                for ci in range(2):
                    nc.tensor.matmul(
                        ps,
                        w_sb[:, ci, co * 128:(co + 1) * 128],
                        x16_sb[:, ci, nsl],
                        start=(ci == 0), stop=(ci == 1),
                    )
                dst_ap = dst[:, co, nsl]
                if nb % 2 == 0:
                    nc.scalar.add(dst_ap, ps, b_sb[:, co:co + 1])
                else:
                    nc.vector.tensor_scalar_add(dst_ap, ps, b_sb[:, co:co + 1])

    # ---- VT[m, c] = sum_ci x[ci, m]^T wvT[ci, c] + bv ----
    vt_sb = P["big"].tile([128, MCH, C], DT_V, tag="vt", name="vt_sb")
    bvap = cst["bv"]
    bv_b = bass.AP(tensor=bvap.tensor, offset=bvap.offset,
                   ap=[bvap.ap[0], [0, 2], bvap.ap[1]])
    vt_rot = ["st", "st", "msg", "proj"]
    for mp in range(MCH // 2):
        rtag = vt_rot[mp % 4]
        ps = P[rtag].tile([128, 2, C], F32, tag=rtag, name="vt_ps")
        for j in range(2):
            mi = 2 * mp + j
            for ci in range(2):
                nc.tensor.matmul(
                    ps[:, j, :],
                    x16_sb[:, ci, mi * 128:(mi + 1) * 128],
                    cst["wvT"][:, ci, :],
                    start=(ci == 0), stop=(ci == 1),
                )
        nc.vector.tensor_add(vt_sb[:, 2 * mp:2 * mp + 2, :], ps, bv_b)

    # ---- attention + MLP per n-block, software-pipelined: block nb's MLP
    # (recip/norm on DVE, h-chain on PE) is emitted a few mi-iterations into
    # block nb+1's attention so the in-order PE is never parked on the DVE
    # tail at a block boundary. ----
    # The previous block's MLP is emitted whole at this block's mi=3
    # (spreading the stages across more iterations measured WORSE on HW).
    MLP_DELAY_MI = 3
    pending_mlp = [None]

    def emit_mlp_stage(mi=None):
        stages = pending_mlp[0]
        if not stages:
            return
        if mi is None or mi == MLP_DELAY_MI:
            while stages:
                stages.pop(0)()
            pending_mlp[0] = None

    for nb in range(NB):
        nsl = slice(nb * BLK, (nb + 1) * BLK)
        msg_ps = P["msg"].tile([128, 2 * BLK], F32, tag="msg", name="msg_ps")
        sums_ps = None
        bts = {}
        for mp in range(MCH // 2):
            bt = P["bt"].tile([128, 2, BLK], beta_dt, tag="bt", name="bt_sb")
            nc.sync.dma_start(
                out=bt,
                in_=betaT_d.ap()[2 * mp * 128:(2 * mp + 2) * 128, nsl]
                    .rearrange("(a p) n -> p a n", p=128))
            bts[mp] = bt
        acc = P["acc"].tile([128, BLK], F32R, tag="acc", name="acc_sb") \
            if pool_set else None
        pool_pending = []
        acc_started = False

        def drain_pool(force=False):
            nonlocal acc_started, pool_pending
            while pool_pending:
                if not acc_started:
                    if len(pool_pending) >= 2:
                        a, b2 = pool_pending[0], pool_pending[1]
                        pool_pending = pool_pending[2:]
                        nc.gpsimd.tensor_tensor(out=acc, in0=a, in1=b2, op=OP.add)
                        acc_started = True
                    elif force:
                        a = pool_pending.pop(0)
                        nc.gpsimd.tensor_copy(out=acc, in_=a)
                        acc_started = True
                    else:
                        return
                else:
                    a = pool_pending.pop(0)
                    nc.gpsimd.tensor_tensor(out=acc, in0=acc, in1=a, op=OP.add)

        # The es-consuming PE matmuls (msg, pe-sums) are emitted one mi later
        # than their producer: the engine wait-queue is only 4 deep, and
        # stacking msg x2 + sums behind a just-issued es stalls the PE
        # sequencer; with a 1-iteration skew the es has ~1us of slack.
        es_pending = []
        sums_pending = []

        def emit_sums(es, mi):
            nonlocal sums_ps
            if sums_ps is None:
                sums_ps = P["proj"].tile([128, BLK], F32, tag="proj",
                                         name="sums_ps")
            nc.tensor.matmul(sums_ps, cst["ones"], es,
                             start=(mi == pe_set[0]),
                             stop=(not pool_set and mi == pe_set[-1]))

        def emit_consumers(es, mi):
            nc.tensor.matmul(msg_ps[:, 0:BLK], vt_sb[:, mi, 0:128], es,
                             start=(mi == 0), stop=(mi == MCH - 1))
            nc.tensor.matmul(msg_ps[:, BLK:2 * BLK], vt_sb[:, mi, 128:256], es,
                             start=(mi == 0), stop=(mi == MCH - 1))
            if mi not in pool_set:
                sums_pending.append((es, mi))
                if len(sums_pending) > sums_skew - 2:
                    emit_sums(*sums_pending.pop(0))

        for mi in range(MCH):
            emit_mlp_stage(mi)
            msl = slice(mi * 128, (mi + 1) * 128)
            st = P["st"].tile([128, BLK], F32, tag="st", name="st_ps")
            for ci in range(2):
                nc.tensor.matmul(
                    st,
                    k_sb[:, ci, msl],
                    q_sb[:, ci, nsl],
                    start=(ci == 0), stop=(ci == 1),
                )
            sbm = P["sbm"].tile([128, BLK], F32, tag="sbm", name="sbm_sb")
            nc.vector.tensor_mul(sbm, st, bts[mi // 2][:, mi % 2, :])
            es = P["es"].tile([128, BLK], DT_V, tag="es", name="es_sb")
            nc.scalar.activation(es, sbm, AF.Exp)
            es_pending.append((es, mi))
            if len(es_pending) > es_skew:
                emit_consumers(*es_pending.pop(0))
            if mi in pool_set:
                pool_pending.append(es)
                drain_pool()
                if mi == pool_sums - 1:
                    drain_pool(force=True)
        while es_pending:
            emit_consumers(*es_pending.pop(0))
        while sums_pending:
            emit_sums(*sums_pending.pop(0))
        # fold the Pool accumulator into the sums bank last: by the time the
        # in-order PE reaches this, the Pool chain has long finished
        if pool_set:
            if sums_ps is None:
                sums_ps = P["proj"].tile([128, BLK], F32, tag="proj",
                                         name="sums_ps")
            nc.tensor.matmul(sums_ps, cst["ones32"], acc,
                             start=(not pe_set), stop=True)
        def make_mlp(nsl, msg_ps, sums_ps, splits=1):
            # MLP: h1 = relu(w1f@msg+b1f); h2 = relu(w2f@h1+b2f);
            # out = (x+b3) + w3@h2. For the final block (nothing left to
            # overlap with), run in column halves pipelined across the
            # proj and msg PSUM banks to hide the relu handoffs.
            W = BLK // splits
            hs = [(s, slice(s * W, (s + 1) * W),
                   "proj" if s % 2 == 0 else "msg") for s in range(splits)]
            st8 = {}

            def stage_norm():
                recip = P["recip"].tile([128, BLK], F32, tag="recip",
                                        name="recip_sb")
                nc.vector.reciprocal(recip, sums_ps)
                msg_sb = P["msgsb"].tile([128, 2, BLK], DT_M, tag="msgsb",
                                         name="msg_sb")
                nc.vector.tensor_mul(msg_sb[:, 0, :], msg_ps[:, 0:BLK], recip)
                nc.vector.tensor_mul(msg_sb[:, 1, :], msg_ps[:, BLK:2 * BLK],
                                     recip)
                st8["msg_sb"] = msg_sb

            def stage_h1():
                for s, csl, pool in hs:
                    h1p = P[pool].tile([128, W], F32, tag=pool, name="h1_ps")
                    for ci in range(2):
                        nc.tensor.matmul(h1p, cst["w1T"][:, ci, :],
                                         st8["msg_sb"][:, ci, csl],
                                         start=(ci == 0), stop=(ci == 1))
                    h1 = P["h"].tile([128, W], DT_M, tag=f"h1{s}",
                                     name="h1_sb")
                    nc.scalar.activation(h1, h1p, AF.Relu,
                                         bias=cst["b1"][:, 0:1])
                    st8[f"h1{s}"] = h1

            def stage_h2():
                for s, csl, pool in hs:
                    h2p = P[pool].tile([128, W], F32, tag=pool, name="h2_ps")
                    nc.tensor.matmul(h2p, cst["w2T"], st8[f"h1{s}"],
                                     start=True, stop=True)
                    h2 = P["h"].tile([128, W], DT_M, tag=f"h2{s}",
                                     name="h2_sb")
                    nc.scalar.activation(h2, h2p, AF.Relu,
                                         bias=cst["b2"][:, 0:1])
                    st8[f"h2{s}"] = h2

            def stage_h3():
                base = nsl.start
                for s, csl, pool in hs:
                    for co in range(2):
                        h3p = P[pool].tile([128, W], F32, tag=pool,
                                           name="h3_ps")
                        xsl = slice(base + csl.start, base + csl.stop)
                        # seed the accumulator with the prefolded residual
                        # (x + b3) via an identity matmul, then accumulate
                        # w3@h2 on top
                        nc.tensor.matmul(h3p, cst["ident"],
                                         x_sb[:, co, xsl],
                                         start=True, stop=False)
                        nc.tensor.matmul(h3p,
                                         cst["w3T"][:, co * 128:(co + 1) * 128],
                                         st8[f"h2{s}"], start=False, stop=True,
                                         skip_group_check=True)
                        ob = P["outp"].tile([128, W], F32, tag="ob",
                                            name="ob_sb")
                        nc.scalar.activation(ob, h3p, AF.Copy)
                        nc.sync.dma_start(
                            out=out_d.ap()[co * 128:(co + 1) * 128, xsl],
                            in_=ob)

            return [stage_norm, stage_h1, stage_h2, stage_h3]

        pending_mlp[0] = make_mlp(nsl, msg_ps, sums_ps,
                                  splits=(2 if nb == NB - 1 else 1))
    emit_mlp_stage()


def _prep_host(inputs, beta_dt=F16, **_kw):
    """Fold BN into conv weights, pre-transpose weights, build per-core maps."""
    f = np.float32
    wq, bq = inputs["wq"].astype(f), inputs["bq"].astype(f)
    wk, bk = inputs["wk"].astype(f), inputs["bk"].astype(f)
    wv, bv = inputs["wv"].astype(f), inputs["bv"].astype(f)
    inv1 = inputs["g1"] / np.sqrt(inputs["v1"] + EPS)
    w1f = (inputs["w1"] * inv1[:, None]).astype(f)
    b1f = (inputs["b1"] * inv1 + inputs["be1"] - inputs["m1"] * inv1).astype(f)
    inv2 = inputs["g2"] / np.sqrt(inputs["v2"] + EPS)
    w2f = (inputs["w2"] * inv2[:, None]).astype(f)
    b2f = (inputs["b2"] * inv2 + inputs["be2"] - inputs["m2"] * inv2).astype(f)
    w3, b3 = inputs["w3"].astype(f), inputs["b3"].astype(f)

    def fold2(wT):  # [256, X] -> [128, 2*X] with t-major columns
        X = wT.shape[1]
        return wT.reshape(2, 128, X).transpose(1, 0, 2).reshape(128, 2 * X)

    lay4, lay16, n4, n16 = _pack_layout()
    pack4 = np.zeros((128, n4), dtype=f)
    pack16 = np.zeros((128, n16), dtype=ml_dtypes.bfloat16)

    def put(name, arr):
        if name in lay4:
            off, ncols = lay4[name]
            pack4[:, off:off + ncols] = arr
        else:
            off, ncols = lay16[name]
            pack16[:, off:off + ncols] = arr.astype(ml_dtypes.bfloat16)

    put("wqT", fold2(wq.T))
    put("wkT", fold2(wk.T))
    put("wvT", fold2(wv.T))
    put("w1T", fold2(w1f.T))
    put("w2T", w2f.T)
    put("w3T", w3.T)
    bias_cols = np.zeros((128, 8), dtype=f)
    bias_cols[:, 0:2] = bq.reshape(2, 128).T
    bias_cols[:, 2:4] = bk.reshape(2, 128).T
    bias_cols[:, 6] = b1f
    bias_cols[:, 7] = b2f
    put("bias", bias_cols)
    put("bv", np.tile(bv, (128, 1)))
    put("ones", np.ones((128, 128), dtype=f))
    put("ones32", np.ones((128, 128), dtype=f))
    put("ident", np.eye(128, dtype=f))
    shared = {"wpack": pack4}
    if lay16:
        shared["wpack16"] = pack16
    x = np.asarray(inputs["cors_feature"], dtype=f)
    beta = np.asarray(inputs["beta_attention"], dtype=f)
    np_beta = {F16: np.float16, BF16: ml_dtypes.bfloat16, F32: f}[beta_dt]
    in_maps = []
    for b in range(B):
        m = dict(shared)
        m["x"] = np.ascontiguousarray(x[b] + b3[:, None])
        m["x16"] = np.ascontiguousarray(x[b]).astype(ml_dtypes.bfloat16)
        m["betaT"] = np.ascontiguousarray(beta[b].T).astype(np_beta)
        in_maps.append(m)
    return in_maps


def kernel(**inputs) -> np.ndarray:
    if "nc" not in _CACHE:
        _CACHE["nc"] = build_nc()
    nc = _CACHE["nc"]
    in_maps = _prep_host(inputs)
    res = bass_utils.run_bass_kernel_spmd(
        nc, in_maps, core_ids=list(range(B)), trace=False)
    out = np.stack([res.results[b]["out"] for b in range(B)], axis=0)
    return out.astype(np.float32)


# revision 95
# speedup vs baseline: 1.0834x; 1.0412x over previous
"""Trainium2 Bass kernel for NonlocalSingleBlock (B=8, C=256, N=2048).

Sharding: data-parallel over batch B across the 8 NeuronCores (one batch
element per core). Per core:
  Q = wq@x+bq, K = wk@x+bk (natural [C,N] layout)
  VT = (wv@x+bv)^T computed directly as x^T @ wv^T (no on-chip transpose)
  S^T tiles [m,n] = K^T Q; scaled by host-pretransposed beta^T; exp on ACT
  (softmax without max-subtraction -- |S*beta| is bounded well under exp
  overflow)
  Softmax row-sums: split between the PE (ones-matmul over a subset of the
  exp'd score tiles) and the otherwise-idle GPSIMD/Pool engine (elementwise
  accumulation chain over the rest, finished by one f32r ones-matmul).
  message = VT-stationary matmuls; MLP with BatchNorm folded into the conv
  weights host-side; residual add.
Attention/MLP matmuls run in bf16; QKV projections stay f32r for accuracy.
beta^T streams as fp16 (half the DMA of f32 at ~1e-4 relative error -- the
fp32 max-|S*beta| exponent error this introduces is ~0.03).
All weights are packed into one DRAM image (single DMA). The softmax
pipeline is 4-deep double-buffered (PSUM: st x4, msg x2 banks, sums, proj)
so PE overlaps the DVE multiply and ACT exp stages.
"""

import numpy as np
import ml_dtypes

import concourse.bass as bass
import concourse.bacc as bacc
import concourse.tile as tile
import concourse.mybir as mybir
import concourse.bass_utils as bass_utils

B, C, N = 8, 256, 2048
EPS = 1e-5
F32 = mybir.dt.float32
F32R = mybir.dt.float32r
BF16 = mybir.dt.bfloat16
F16 = mybir.dt.float16
NB = 4          # n-blocks per core
BLK = N // NB   # 512 query columns per block
MCH = N // 128  # 16 key chunks of 128

DT_S = BF16     # Q/K/S^T matmul dtype
DT_V = BF16     # es/VT path
DT_M = BF16     # MLP path
DT_X = F32R     # x for QKV projections

_CACHE = {}


def _np_of(dt):
    return ml_dtypes.bfloat16 if dt == BF16 else np.float32


def _pack_layout():
    """Column layout of the packed weight images. Returns (lay4, lay16, n4,
    n16) where lay*[name] = (start, ncols). wqT/wkT lead the bf16 image so
    the startup head-DMA covers exactly what the first matmuls need."""
    entries = [
        ("wqT", 512, BF16), ("wkT", 512, BF16), ("wvT", 512, BF16),
        ("w1T", 256, DT_M), ("w2T", 128, DT_M), ("w3T", 256, DT_M),
        ("ones", 128, DT_V),
        ("bias", 8, F32R), ("bv", 256, F32R), ("ones32", 128, F32R),
        ("ident", 128, F32R),
    ]
    lay4, lay16 = {}, {}
    n4 = n16 = 0
    for name, ncols, dt in entries:
        if dt == BF16:
            lay16[name] = (n16, ncols)
            n16 += ncols
        else:
            lay4[name] = (n4, ncols)
            n4 += ncols
    return lay4, lay16, n4, max(n16, 1)


def build_nc(loop_iters=None, beta_dt=F16, pool_sums=12, qk_bias="mixed",
             bt_bufs=8, outp_bufs=4, sums_skew=2, es_skew=0, q_v3=False):
    nc = bacc.Bacc("TRN2", target_bir_lowering=False, debug=False)

    d = {}
    # "x" carries the host-prefolded residual x + b3 (f32 bits read as f32r)
    d["x"] = nc.dram_tensor("x", [C, N], F32R, kind="ExternalInput")
    d["x16"] = nc.dram_tensor("x16", [C, N], BF16, kind="ExternalInput")
    d["betaT"] = nc.dram_tensor("betaT", [N, N], beta_dt, kind="ExternalInput")
    lay4, lay16, n4, n16 = _pack_layout()
    d["wpack"] = nc.dram_tensor("wpack", [128, n4], F32R, kind="ExternalInput")
    if lay16:
        d["wpack16"] = nc.dram_tensor("wpack16", [128, n16], BF16,
                                      kind="ExternalInput")
    d["out"] = nc.dram_tensor("out", [C, N], F32, kind="ExternalOutput")
    if loop_iters == "rt":
        # runtime trip count: one executable serves every K, so the
        # per-executable dispatch floor cancels exactly in differentials
        d["niter"] = nc.dram_tensor("niter", [1, 1], mybir.dt.int32,
                                    kind="ExternalInput")

    from contextlib import ExitStack, nullcontext
    with tile.TileContext(nc) as tc, ExitStack() as ctx:
        P = {}
        P["consts"] = ctx.enter_context(tc.tile_pool(name="consts", bufs=1))
        P["big"] = ctx.enter_context(tc.tile_pool(name="big", bufs=1))
        P["bt"] = ctx.enter_context(tc.tile_pool(name="bt", bufs=bt_bufs))
        P["es"] = ctx.enter_context(tc.tile_pool(name="es", bufs=10))
        P["sbm"] = ctx.enter_context(tc.tile_pool(name="sbm", bufs=4))
        P["acc"] = ctx.enter_context(tc.tile_pool(name="acc", bufs=2))
        P["msgsb"] = ctx.enter_context(tc.tile_pool(name="msgsb", bufs=2))
        P["recip"] = ctx.enter_context(tc.tile_pool(name="recip", bufs=2))
        P["h"] = ctx.enter_context(tc.tile_pool(name="h", bufs=2))
        P["outp"] = ctx.enter_context(tc.tile_pool(name="outp", bufs=outp_bufs))
        # PSUM banks: st 3x1 + msg 2x2 + proj 1 = 8 (softmax sums share the
        # proj bank -- they complete right before the MLP needs it)
        P["st"] = ctx.enter_context(tc.tile_pool(name="st", bufs=3, space="PSUM"))
        P["msg"] = ctx.enter_context(tc.tile_pool(name="msg", bufs=2, space="PSUM"))
        P["proj"] = ctx.enter_context(tc.tile_pool(name="proj", bufs=1, space="PSUM"))

        cst = _load_consts(nc, P, d, q_v3)
        cst["q_v3"] = q_v3
        if loop_iters == "rt":
            nit_sb = P["consts"].tile([1, 1], mybir.dt.int32, name="nit_sb")
            nc.sync.dma_start(out=nit_sb, in_=d["niter"].ap())
            regs = []
            for eng in nc.engines.values():
                r = eng.alloc_register(f"niter_{eng.engine.name}")
                eng.reg_load(r, nit_sb)
                regs.append(r)
            bound = nc.snap(bass.RegisterHandles(regs), min_val=1,
                            max_val=1 << 20)
            loop_cm = tc.For_i(0, bound, 1)
        else:
            loop_cm = tc.For_i(0, loop_iters, 1) if loop_iters \
                else nullcontext()
        with loop_cm:
            _emit_body(nc, tc, P, d, cst, beta_dt, pool_sums, qk_bias,
                       sums_skew, es_skew)

    nc.compile()
    return nc


def _load_consts(nc, P, d, q_v3=False):
    consts = P["consts"]
    lay4, lay16, n4, n16 = _pack_layout()
    cst = {}
    # Startup DMAs fan out across queues so the serial per-DMA overhead
    # (~1.4us each) isn't stacked on one queue: scalar gets the wqT/wkT head
    # (first matmuls) then the f32 x; sync gets the small f32 pack (biases)
    # and the rest of the bf16 pack (wvT for the VT phase) before the beta
    # stream; the x16 chunks ride the vector queue.
    wp16 = consts.tile([128, n16], BF16, name="wp16_sb")
    head = min(1024, n16)
    nc.scalar.dma_start(out=wp16[:, 0:head], in_=d["wpack16"].ap()[:, 0:head])
    wp4 = consts.tile([128, n4], F32R, name="wp4_sb")
    deferred_eng = nc.sync if q_v3 else nc.scalar
    cst["_deferred"] = lambda: (
        deferred_eng.dma_start(out=wp4, in_=d["wpack"].ap()),
        deferred_eng.dma_start(out=wp16[:, head:n16],
                               in_=d["wpack16"].ap()[:, head:n16])
        if n16 > head else None,
    )

    def sl(name):
        lay, t = (lay4, wp4) if name in lay4 else (lay16, wp16)
        off, ncols = lay[name]
        return t[:, off:off + ncols]

    for nm in ("wqT", "wkT", "wvT"):
        cst[nm] = sl(nm).rearrange("p (t o) -> p t o", t=2)
    cst["w1T"] = sl("w1T").rearrange("p (t o) -> p t o", t=2)
    cst["w2T"] = sl("w2T")
    cst["w3T"] = sl("w3T")
    b = sl("bias").bitcast(F32)
    cst["bq"] = b[:, 0:2]
    cst["bk"] = b[:, 2:4]
    cst["b3"] = b[:, 4:6]
    cst["b1"] = b[:, 6:7]
    cst["b2"] = b[:, 7:8]
    cst["bv"] = sl("bv").bitcast(F32)
    cst["ones"] = sl("ones")
    cst["ones32"] = sl("ones32")
    cst["ident"] = sl("ident")
    return cst


def _emit_body(nc, tc, P, d, cst, beta_dt, pool_sums, qk_bias, sums_skew=2,
               es_skew=2):
    AF = mybir.ActivationFunctionType
    OP = mybir.AluOpType
    x_d, betaT_d, out_d = d["x"], d["betaT"], d["out"]

    # First pool_sums mi chunks' es tiles are accumulated on the Pool engine
    # (prefix, so the chain and its closing f32r ones-matmul finish mid-block
    # instead of serializing the block tail); the rest go through the PE
    # ones-matmul as before.
    pool_set = set(range(pool_sums))
    pe_set = [mi for mi in range(MCH) if mi not in pool_set]

    # ---- x16 (4 DMAs on the scalar queue, parallel to the wpack DMAs on
    # the sync queue, so the first QK matmuls start early). The f32 x for
    # the residual follows on the sync queue -- it is not needed until the
    # first MLP, ~20us in. ----
    # x16 chunk 0 rides the sync queue in parallel with the wqT/wkT head on
    # the scalar queue, so the first matmul can start after ~one DMA latency;
    # the remaining chunks stream on scalar behind the head.
    x16_sb = P["big"].tile([128, 2, N], BF16, tag="x16", name="x16_sb")
    x16_re = d["x16"].ap().rearrange("(t p) n -> p t n", p=128)
    qs = slice(0, N // 4)
    (nc.sync if cst["q_v3"] else nc.scalar).dma_start(
        out=x16_sb[:, :, qs], in_=x16_re[:, :, qs])
    cst["_deferred"]()  # small f32 pack + rest of bf16 pack, needed ~5us in
    for q in range(1, 4):
        qs = slice(q * (N // 4), (q + 1) * (N // 4))
        nc.scalar.dma_start(out=x16_sb[:, :, qs], in_=x16_re[:, :, qs])
    x_sb = P["big"].tile([128, 2, N], F32R, tag="x", name="x_sb")
    nc.scalar.dma_start(out=x_sb,
                        in_=x_d.ap().rearrange("(t p) n -> p t n", p=128))

    # ---- Q, K: [co, n] = sum_ci wT[ci, co] x[ci, n] + b[co] ----
    q_sb = P["big"].tile([128, 2, N], DT_S, tag="q", name="q_sb")
    k_sb = P["big"].tile([128, 2, N], DT_S, tag="k", name="k_sb")
    qk_rot = ["st", "st", "st", "msg", "msg", "proj"]
    qk_i = 0
    for w_sb, b_sb, dst in ((cst["wqT"], cst["bq"], q_sb), (cst["wkT"], cst["bk"], k_sb)):
        for co in range(2):
            for nb in range(NB):
                rtag = qk_rot[qk_i % len(qk_rot)]
                qk_i += 1
                ps = P[rtag].tile([128, BLK], F32, tag=rtag, name="qk_ps")
                nsl = slice(nb * BLK, (nb + 1) * BLK)
                for ci in range(2):
                    nc.tensor.matmul(
                        ps,
                        w_sb[:, ci, co * 128:(co + 1) * 128],
                        x16_sb[:, ci, nsl],
                        start=(ci == 0), stop=(ci == 1),
                    )
                dst_ap = dst[:, co, nsl]
                if nb % 2 == 0:
                    nc.scalar.add(dst_ap, ps, b_sb[:, co:co + 1])
                else:
                    nc.vector.tensor_scalar_add(dst_ap, ps, b_sb[:, co:co + 1])

    # ---- VT[m, c] = sum_ci x[ci, m]^T wvT[ci, c] + bv ----
    vt_sb = P["big"].tile([128, MCH, C], DT_V, tag="vt", name="vt_sb")
    bvap = cst["bv"]
    bv_b = bass.AP(tensor=bvap.tensor, offset=bvap.offset,
                   ap=[bvap.ap[0], [0, 2], bvap.ap[1]])
    vt_rot = ["st", "st", "msg", "proj"]
    for mp in range(MCH // 2):
        rtag = vt_rot[mp % 4]
        ps = P[rtag].tile([128, 2, C], F32, tag=rtag, name="vt_ps")
        for j in range(2):
            mi = 2 * mp + j
            for ci in range(2):
                nc.tensor.matmul(
                    ps[:, j, :],
                    x16_sb[:, ci, mi * 128:(mi + 1) * 128],
                    cst["wvT"][:, ci, :],
                    start=(ci == 0), stop=(ci == 1),
                )
        nc.vector.tensor_add(vt_sb[:, 2 * mp:2 * mp + 2, :], ps, bv_b)

    # ---- attention + MLP per n-block, software-pipelined: block nb's MLP
    # (recip/norm on DVE, h-chain on PE) is emitted a few mi-iterations into
    # block nb+1's attention so the in-order PE is never parked on the DVE
    # tail at a block boundary. ----
    # The previous block's MLP is emitted whole at this block's mi=3
    # (spreading the stages across more iterations measured WORSE on HW).
    MLP_DELAY_MI = 3
    pending_mlp = [None]

    def emit_mlp_stage(mi=None):
        stages = pending_mlp[0]
        if not stages:
            return
        if mi is None or mi == MLP_DELAY_MI:
            while stages:
                stages.pop(0)()
            pending_mlp[0] = None

    for nb in range(NB):
        nsl = slice(nb * BLK, (nb + 1) * BLK)
        msg_ps = P["msg"].tile([128, 2 * BLK], F32, tag="msg", name="msg_ps")
        sums_ps = None
        bts = {}
        for mp in range(MCH // 2):
            bt = P["bt"].tile([128, 2, BLK], beta_dt, tag="bt", name="bt_sb")
            nc.sync.dma_start(
                out=bt,
                in_=betaT_d.ap()[2 * mp * 128:(2 * mp + 2) * 128, nsl]
                    .rearrange("(a p) n -> p a n", p=128))
            bts[mp] = bt
        acc = P["acc"].tile([128, BLK], F32R, tag="acc", name="acc_sb") \
            if pool_set else None
        pool_pending = []
        acc_started = False

        def drain_pool(force=False):
            nonlocal acc_started, pool_pending
            while pool_pending:
                if not acc_started:
                    if len(pool_pending) >= 2:
                        a, b2 = pool_pending[0], pool_pending[1]
                        pool_pending = pool_pending[2:]
                        nc.gpsimd.tensor_tensor(out=acc, in0=a, in1=b2, op=OP.add)
                        acc_started = True
                    elif force:
                        a = pool_pending.pop(0)
                        nc.gpsimd.tensor_copy(out=acc, in_=a)
                        acc_started = True
                    else:
                        return
                else:
                    a = pool_pending.pop(0)
                    nc.gpsimd.tensor_tensor(out=acc, in0=acc, in1=a, op=OP.add)

        # The es-consuming PE matmuls (msg, pe-sums) are emitted one mi later
        # than their producer: the engine wait-queue is only 4 deep, and
        # stacking msg x2 + sums behind a just-issued es stalls the PE
        # sequencer; with a 1-iteration skew the es has ~1us of slack.
        es_pending = []
        sums_pending = []

        def emit_sums(es, mi):
            nonlocal sums_ps
            if sums_ps is None:
                sums_ps = P["proj"].tile([128, BLK], F32, tag="proj",
                                         name="sums_ps")
            nc.tensor.matmul(sums_ps, cst["ones"], es,
                             start=(mi == pe_set[0]),
                             stop=(not pool_set and mi == pe_set[-1]))

        def emit_consumers(es, mi):
            nc.tensor.matmul(msg_ps[:, 0:BLK], vt_sb[:, mi, 0:128], es,
                             start=(mi == 0), stop=(mi == MCH - 1))
            nc.tensor.matmul(msg_ps[:, BLK:2 * BLK], vt_sb[:, mi, 128:256], es,
                             start=(mi == 0), stop=(mi == MCH - 1))
            if mi not in pool_set:
                sums_pending.append((es, mi))
                if len(sums_pending) > sums_skew - 2:
                    emit_sums(*sums_pending.pop(0))

        for mi in range(MCH):
            emit_mlp_stage(mi)
            msl = slice(mi * 128, (mi + 1) * 128)
            st = P["st"].tile([128, BLK], F32, tag="st", name="st_ps")
            for ci in range(2):
                nc.tensor.matmul(
                    st,
                    k_sb[:, ci, msl],
                    q_sb[:, ci, nsl],
                    start=(ci == 0), stop=(ci == 1),
                )
            sbm = P["sbm"].tile([128, BLK], F32, tag="sbm", name="sbm_sb")
            nc.vector.tensor_mul(sbm, st, bts[mi // 2][:, mi % 2, :])
            es = P["es"].tile([128, BLK], DT_V, tag="es", name="es_sb")
            nc.scalar.activation(es, sbm, AF.Exp)
            es_pending.append((es, mi))
            if len(es_pending) > es_skew:
                emit_consumers(*es_pending.pop(0))
            if mi in pool_set:
                pool_pending.append(es)
                drain_pool()
                if mi == pool_sums - 1:
                    drain_pool(force=True)
        while es_pending:
            emit_consumers(*es_pending.pop(0))
        while sums_pending:
            emit_sums(*sums_pending.pop(0))
        # fold the Pool accumulator into the sums bank last: by the time the
        # in-order PE reaches this, the Pool chain has long finished
        if pool_set:
            if sums_ps is None:
                sums_ps = P["proj"].tile([128, BLK], F32, tag="proj",
                                         name="sums_ps")
            nc.tensor.matmul(sums_ps, cst["ones32"], acc,
                             start=(not pe_set), stop=True)
        def make_mlp(nsl, msg_ps, sums_ps, splits=1):
            # MLP: h1 = relu(w1f@msg+b1f); h2 = relu(w2f@h1+b2f);
            # out = (x+b3) + w3@h2. For the final block (nothing left to
            # overlap with), run in column halves pipelined across the
            # proj and msg PSUM banks to hide the relu handoffs.
            W = BLK // splits
            hs = [(s, slice(s * W, (s + 1) * W),
                   "proj" if s % 2 == 0 else "msg") for s in range(splits)]
            st8 = {}

            def stage_norm():
                recip = P["recip"].tile([128, BLK], F32, tag="recip",
                                        name="recip_sb")
                nc.vector.reciprocal(recip, sums_ps)
                msg_sb = P["msgsb"].tile([128, 2, BLK], DT_M, tag="msgsb",
                                         name="msg_sb")
                nc.vector.tensor_mul(msg_sb[:, 0, :], msg_ps[:, 0:BLK], recip)
                nc.vector.tensor_mul(msg_sb[:, 1, :], msg_ps[:, BLK:2 * BLK],
                                     recip)
                st8["msg_sb"] = msg_sb

            def stage_h1():
                for s, csl, pool in hs:
                    h1p = P[pool].tile([128, W], F32, tag=pool, name="h1_ps")
                    for ci in range(2):
                        nc.tensor.matmul(h1p, cst["w1T"][:, ci, :],
                                         st8["msg_sb"][:, ci, csl],
                                         start=(ci == 0), stop=(ci == 1))
                    h1 = P["h"].tile([128, W], DT_M, tag=f"h1{s}",
                                     name="h1_sb")
                    nc.scalar.activation(h1, h1p, AF.Relu,
                                         bias=cst["b1"][:, 0:1])
                    st8[f"h1{s}"] = h1

            def stage_h2():
                for s, csl, pool in hs:
                    h2p = P[pool].tile([128, W], F32, tag=pool, name="h2_ps")
                    nc.tensor.matmul(h2p, cst["w2T"], st8[f"h1{s}"],
                                     start=True, stop=True)
                    h2 = P["h"].tile([128, W], DT_M, tag=f"h2{s}",
                                     name="h2_sb")
                    nc.scalar.activation(h2, h2p, AF.Relu,
                                         bias=cst["b2"][:, 0:1])
                    st8[f"h2{s}"] = h2

            def stage_h3():
                base = nsl.start
                for s, csl, pool in hs:
                    for co in range(2):
                        h3p = P[pool].tile([128, W], F32, tag=pool,
                                           name="h3_ps")
                        xsl = slice(base + csl.start, base + csl.stop)
                        # seed the accumulator with the prefolded residual
                        # (x + b3) via an identity matmul, then accumulate
                        # w3@h2 on top
                        nc.tensor.matmul(h3p, cst["ident"],
                                         x_sb[:, co, xsl],
                                         start=True, stop=False)
                        nc.tensor.matmul(h3p,
                                         cst["w3T"][:, co * 128:(co + 1) * 128],
                                         st8[f"h2{s}"], start=False, stop=True,
                                         skip_group_check=True)
                        ob = P["outp"].tile([128, W], F32, tag="ob",
                                            name="ob_sb")
                        nc.scalar.activation(ob, h3p, AF.Copy)
                        oeng = nc.scalar if (cst["q_v3"] and co == 1) \
                            else nc.sync
                        oeng.dma_start(
                            out=out_d.ap()[co * 128:(co + 1) * 128, xsl],
                            in_=ob)

            return [stage_norm, stage_h1, stage_h2, stage_h3]

        pending_mlp[0] = make_mlp(nsl, msg_ps, sums_ps,
                                  splits=(2 if nb == NB - 1 else 1))
    emit_mlp_stage()


def _prep_host(inputs, beta_dt=F16, **_kw):
    """Fold BN into conv weights, pre-transpose weights, build per-core maps."""
    f = np.float32
    wq, bq = inputs["wq"].astype(f), inputs["bq"].astype(f)
    wk, bk = inputs["wk"].astype(f), inputs["bk"].astype(f)
    wv, bv = inputs["wv"].astype(f), inputs["bv"].astype(f)
    inv1 = inputs["g1"] / np.sqrt(inputs["v1"] + EPS)
    w1f = (inputs["w1"] * inv1[:, None]).astype(f)
    b1f = (inputs["b1"] * inv1 + inputs["be1"] - inputs["m1"] * inv1).astype(f)
    inv2 = inputs["g2"] / np.sqrt(inputs["v2"] + EPS)
    w2f = (inputs["w2"] * inv2[:, None]).astype(f)
    b2f = (inputs["b2"] * inv2 + inputs["be2"] - inputs["m2"] * inv2).astype(f)
    w3, b3 = inputs["w3"].astype(f), inputs["b3"].astype(f)

    def fold2(wT):  # [256, X] -> [128, 2*X] with t-major columns
        X = wT.shape[1]
        return wT.reshape(2, 128, X).transpose(1, 0, 2).reshape(128, 2 * X)

    lay4, lay16, n4, n16 = _pack_layout()
    pack4 = np.zeros((128, n4), dtype=f)
    pack16 = np.zeros((128, n16), dtype=ml_dtypes.bfloat16)

    def put(name, arr):
        if name in lay4:
            off, ncols = lay4[name]
            pack4[:, off:off + ncols] = arr
        else:
            off, ncols = lay16[name]
            pack16[:, off:off + ncols] = arr.astype(ml_dtypes.bfloat16)

    put("wqT", fold2(wq.T))
    put("wkT", fold2(wk.T))
    put("wvT", fold2(wv.T))
    put("w1T", fold2(w1f.T))
    put("w2T", w2f.T)
    put("w3T", w3.T)
    bias_cols = np.zeros((128, 8), dtype=f)
    bias_cols[:, 0:2] = bq.reshape(2, 128).T
    bias_cols[:, 2:4] = bk.reshape(2, 128).T
    bias_cols[:, 6] = b1f
    bias_cols[:, 7] = b2f
    put("bias", bias_cols)
    put("bv", np.tile(bv, (128, 1)))
    put("ones", np.ones((128, 128), dtype=f))
    put("ones32", np.ones((128, 128), dtype=f))
    put("ident", np.eye(128, dtype=f))
    shared = {"wpack": pack4}
    if lay16:
        shared["wpack16"] = pack16
    x = np.asarray(inputs["cors_feature"], dtype=f)
    beta = np.asarray(inputs["beta_attention"], dtype=f)
    np_beta = {F16: np.float16, BF16: ml_dtypes.bfloat16, F32: f}[beta_dt]
    in_maps = []
    for b in range(B):
        m = dict(shared)
        m["x"] = np.ascontiguousarray(x[b] + b3[:, None])
        m["x16"] = np.ascontiguousarray(x[b]).astype(ml_dtypes.bfloat16)
        m["betaT"] = np.ascontiguousarray(beta[b].T).astype(np_beta)
        in_maps.append(m)
    return in_maps


def kernel(**inputs) -> np.ndarray:
    if "nc" not in _CACHE:
        _CACHE["nc"] = build_nc()
    nc = _CACHE["nc"]
    in_maps = _prep_host(inputs)
    res = bass_utils.run_bass_kernel_spmd(
        nc, in_maps, core_ids=list(range(B)), trace=False)
    out = np.stack([res.results[b]["out"] for b in range(B)], axis=0)
    return out.astype(np.float32)
